# revision 10
# baseline (speedup 1.0000x reference)
"""Trainium2 Bass kernel for nn_ExtractPatchesPositionLayer.

Reference semantics: per image b, bilinear-translate the (522,522,1) padded
object by t = -positions[b] (tfa.translate: out(y,x) = img(y+py, x+px),
zero fill outside), then center-crop 5px -> (512,512,1).

Because the shift is constant per image, floor/frac of the offset give an
integer window start (A,B) into the (zero-margin-padded) image plus four
constant bilinear corner weights; with the host pre-scaling each image by
S=(1-wx)(1-wy), the ratio form (rx=wx/(1-wx), ry=wy/(1-wy)) needs no final
scale and all fp16 intermediates are bounded by max|x| (S*(1+rx)(1+ry)=1).
fp16 I/O costs ~4e-4 rel err vs the 2e-2 gate and halves HBM traffic.

IMAGE PAIRING: one tile holds TWO images -- partitions 0-63 carry image 2t
(8 consecutive rows per partition), partitions 64-127 carry image 2t+1.
Per-partition scalars (rx/ry f32 APs, [128,1]) switch image at partition 64
for free.  This doubles DMA packet size (8.4 KB in / 8 KB out, ~26 GB/s per
SDMA engine vs 23 at 4 KB), halves every per-instruction overhead, and the
pair's output block is CONTIGUOUS in DRAM (one SWDGE descriptor set covers
both images: partition 64's chunk starts exactly at image 2t+1's base).

Engine split (DVE stt has NO fast modes -- 1 elem/cyc; TensorTensor runs
2x_1p on packed fp16, TensorScalarPtr runs 4x_2p on all-SBUF fp16;
verified on HW from the per-instruction trace):
  ACT  t0c = rx*wt[+1]   strided 3D span read, COMPACT write (f32 scale AP)
  DVE  gc  = wt[3D] + t0c            TensorTensor 2x
  PE   ps  = shift @ gc[:,0:N]       boundary rows (cols 63,127 zero)
  ACT  t1r3 = ry*ps                  PSUM read
  DVE  t1c[0:7N] = ry*gc[N:8N]       TensorScalarPtr 4x
  DVE  oc  = gc + t1c                TensorTensor 2x, all flat
Software-pipelined with a 2-stage skew so no engine waits on an
intra-iteration cross-engine round trip.  Rows 511 of each image (boundary
partitions 63/127 have no next partition) are patched on host.

DMA routing (hard-won trace facts): inputs via dynamic HWDGE on the SP ring
(descriptors spread over all 16 SDMA engines by dest partition); outputs
via SWDGE/gpsimd (HWDGE serializes all SBUF->HBM onto SDMA engine 0).
Sharding: batch 256 -> 32 images x 8 cores, embarrassingly parallel.
History: 1426us (banded-matmul PE) -> 181us (f32 stt) -> 178us (fp16 stt,
DVE-bound) -> 119.6us (3-engine split) -> this pairing.
"""

from dataclasses import dataclass

import numpy as np

import concourse.bacc as bacc
import concourse.bass as bass
import concourse.mybir as mybir
import concourse.tile as tile
from concourse.bass_utils import run_bass_kernel_spmd


@dataclass(frozen=True)
class Cfg:
    bpc: int      # images per core (even; processed in pairs)
    n: int        # output height/width
    wpad: int     # padded input height/width (with zero margin)
    xlen: int     # flat padded-input length per core (incl. tail pad)

    @property
    def wrow(self):  # image rows per partition (64 partitions per image)
        return self.n // 64

    @property
    def span(self):  # elements DMA'd per partition (WR rows + 1)
        return self.wrow * self.wpad + 1


def build_nc(cfg: Cfg) -> bass.Bass:
    BPC, N, WPAD = cfg.bpc, cfg.n, cfg.wpad
    WR = cfg.wrow          # 8
    SPAN = cfg.span        # 8*wpad + 1
    XLEN = cfg.xlen
    PAIRS = BPC // 2
    OC = WR * N            # 4096 compact elems per partition
    f32 = mybir.dt.float32
    f16 = mybir.dt.float16
    i32 = mybir.dt.int32

    nc = bacc.Bacc("TRN2", target_bir_lowering=False, debug=False)
    x_d = nc.declare_dram_parameter("x", [1, XLEN], f16, isOutput=False)
    offs_d = nc.declare_dram_parameter("offs", [1, BPC], i32, isOutput=False)
    wmat_d = nc.declare_dram_parameter("wmat", [PAIRS, 128, 4], f32,
                                       isOutput=False)
    smat_d = nc.declare_dram_parameter("smat", [128, 128], f16, isOutput=False)
    y_d = nc.declare_dram_parameter("y", [BPC, N, N], f16, isOutput=True)

    with tile.TileContext(nc) as tc:
        with (
            tc.tile_pool(name="const", bufs=1) as constp,
            tc.tile_pool(name="win", bufs=5) as winp,
            tc.tile_pool(name="t0p", bufs=3) as t0p,
            tc.tile_pool(name="gp", bufs=3) as gp,
            tc.tile_pool(name="tp", bufs=3) as tp,
            tc.tile_pool(name="op", bufs=3) as op,
            tc.tile_pool(name="psp", bufs=8, space="PSUM") as psp,
        ):
            # consts ride the ACT HWDGE ring so the SP ring's FIFO head is
            # the first window DMA (shaves the pipeline ramp)
            wmat_sb = constp.tile([128, PAIRS * 4], f32, tag="wmat")
            nc.scalar.dma_start(
                wmat_sb[:].rearrange("p (i q) -> p i q", q=4),
                wmat_d[:, :, :].transpose([1, 0, 2]),
            )
            offs_sb = constp.tile([1, BPC], i32, tag="offs")
            nc.scalar.dma_start(offs_sb[:], offs_d[:, :])
            smat_sb = constp.tile([128, 128], f16, tag="smat")
            nc.scalar.dma_start(smat_sb[:], smat_d[:, :])

            regs = [nc.alloc_register(mybir.EngineType.SP, f"dynoff_{k}")
                    for k in range(min(16, BPC))]
            svs = [nc.snap(r, donate=True, min_val=0, max_val=XLEN - 1)
                   for r in regs]
            nreg = len(regs)

            st = {}  # pair idx -> dict of live tiles / scalar APs

            def stage_a(t):  # 2 half-tile DMAs + ACT h-lerp mul
                k0, k1 = (2 * t) % nreg, (2 * t + 1) % nreg
                nc.sync.reg_load(regs[k0], offs_sb[0:1, 2 * t: 2 * t + 1])
                nc.sync.reg_load(regs[k1], offs_sb[0:1, 2 * t + 1: 2 * t + 2])
                wt = winp.tile([128, SPAN], f16, tag="wt")
                nc.sync.dma_start(
                    wt[0:64, :],
                    bass.AP(x_d, svs[k0], [[WR * WPAD, 64], [1, SPAN]]),
                )
                nc.sync.dma_start(
                    wt[64:128, :],
                    bass.AP(x_d, svs[k1], [[WR * WPAD, 64], [1, SPAN]]),
                )
                t0c = t0p.tile([128, OC], f16, tag="t0c")
                wt1 = wt[:, 1:WR * WPAD + 1].rearrange(
                    "p (r c) -> p r c", c=WPAD)[:, :, 0:N]
                # scale AP must be FP32 for ACT
                nc.scalar.mul(t0c[:].rearrange("p (r c) -> p r c", c=N),
                              wt1, wmat_sb[:, 4 * t: 4 * t + 1])
                st[t] = {"wt": wt, "t0c": t0c,
                         "ry": wmat_sb[:, 4 * t + 1: 4 * t + 2]}

            def stage_b(t):  # DVE h-lerp add + v-lerp mul, PE boundary rows
                s = st[t]
                wt0 = s["wt"][:, 0:WR * WPAD].rearrange(
                    "p (r c) -> p r c", c=WPAD)[:, :, 0:N]
                gc = gp.tile([128, OC], f16, tag="gc")
                t1c = tp.tile([128, OC], f16, tag="t1c")
                ps = psp.tile([128, N], f32, tag="ps")
                nc.vector.tensor_add(
                    gc[:].rearrange("p (r c) -> p r c", c=N), wt0,
                    s["t0c"][:].rearrange("p (r c) -> p r c", c=N))
                # ps[p,:] = gc[p+1, 0:N] (image row 8p+8); smat cols 63 and
                # 127 all-zero -> rows 511 of both images host-patched
                nc.tensor.matmul(out=ps[:], lhsT=smat_sb[:, :],
                                 rhs=gc[:, 0:N], start=True, stop=True)
                nc.vector.tensor_scalar_mul(
                    t1c[:, 0:(WR - 1) * N], gc[:, N:OC], s["ry"])
                s.update(gc=gc, t1c=t1c, ps=ps)

            def stage_c_act(t):  # ACT: boundary v-lerp term from PSUM
                s = st[t]
                nc.scalar.mul(s["t1c"][:, (WR - 1) * N:OC], s["ps"][:],
                              s["ry"])

            def stage_c_rest(t):  # DVE final add + one SWDGE for the pair
                s = st.pop(t)
                oc = op.tile([128, OC], f16, tag="oc")
                nc.vector.tensor_add(oc[:], s["gc"][:], s["t1c"][:])
                # partition p chunk = y[2t] + p*OC: p=64 lands exactly at
                # y[2t+1]'s base -- the pair is one contiguous DRAM block
                nc.gpsimd.dma_start(
                    bass.AP(y_d, 2 * t * (N * N), [[OC, 128], [1, OC]]),
                    oc[:],
                )

            for t in range(PAIRS + 2):
                if t - 2 >= 0:
                    stage_c_act(t - 2)
                if t < PAIRS:
                    stage_a(t)
                if 0 <= t - 1 < PAIRS:
                    stage_b(t - 1)
                if t - 2 >= 0:
                    stage_c_rest(t - 2)
    nc.compile()
    return nc


def host_prep(padded: np.ndarray, positions: np.ndarray, n_cores: int):
    """Shard + build metadata. padded: (B, npad, npad) f32, positions: (B, 2)."""
    B, npad, _ = padded.shape
    n = npad - 10
    bpc = B // n_cores

    px = positions[:, 0].astype(np.float32)
    py = positions[:, 1].astype(np.float32)
    fy = np.floor(py)
    fx = np.floor(px)
    ay = (5 + fy).astype(np.int64)
    ax = (5 + fx).astype(np.int64)
    wy = (py - fy).astype(np.float32)
    wx = (px - fx).astype(np.float32)

    m_lo = int(max(0, -min(ay.min(), ax.min())))
    m_hi = int(max(0, max(ay.max(), ax.max()) + n + 1 - npad))
    wpad = npad + m_lo + m_hi

    pp = np.zeros((B, wpad, wpad), dtype=np.float32)
    pp[:, m_lo:m_lo + npad, m_lo:m_lo + npad] = padded

    A = ay + m_lo
    Bc = ax + m_lo
    base = (np.arange(B, dtype=np.int64) % bpc) * (wpad * wpad)
    off = base + A * wpad + Bc

    wr = n // 64            # 8 rows per partition, 64 partitions per image
    span = wr * wpad + 1
    # flat length incl. tail so the last image's strided span stays in bounds
    need = int(off.max()) + 63 * wr * wpad + span
    xlen = max(bpc * wpad * wpad, need)

    cfg = Cfg(bpc=bpc, n=n, wpad=wpad, xlen=xlen)

    smat = np.zeros((128, 128), dtype=np.float32)
    for m in range(127):
        if m != 63:  # cols 63/127 zero: rows 511 of each image host-patched
            smat[m + 1, m] = 1.0
    # ps[m, j] = sum_k smat[k, m] g[k, j] = g[m+1, j]

    # host-side fixup for the last output row of each image (needs input row
    # A+n, which the 8-row spans don't load)
    ar = np.arange(B)[:, None]
    ci = Bc[:, None] + np.arange(n + 1)[None, :]
    r0 = pp[ar, (A + n - 1)[:, None], ci]  # (B, n+1)
    r1 = pp[ar, (A + n)[:, None], ci]
    h0r = (1 - wx)[:, None] * r0[:, :n] + wx[:, None] * r0[:, 1:]
    h1r = (1 - wx)[:, None] * r1[:, :n] + wx[:, None] * r1[:, 1:]
    last_row = ((1 - wy)[:, None] * h0r + wy[:, None] * h1r).astype(np.float32)

    # fp16 I/O: pre-scale each image by S=(1-wx)(1-wy) (bounds every
    # ratio-form intermediate by max|x|; no final scale needed)
    S = ((1 - wx) * (1 - wy)).astype(np.float32)
    rx = (wx / (1 - wx)).astype(np.float32)
    ry = (wy / (1 - wy)).astype(np.float32)

    pairs = bpc // 2
    in_maps = []
    for cidx in range(n_cores):
        sl = slice(cidx * bpc, (cidx + 1) * bpc)
        flat = np.zeros((1, xlen), dtype=np.float16)
        flat[0, :bpc * wpad * wpad] = (
            pp[sl] * S[sl][:, None, None]).astype(np.float16).reshape(-1)
        offs = off[sl].astype(np.int32).reshape(1, bpc)
        # pair layout: partitions 0-63 <- image 2t, 64-127 <- image 2t+1
        wmat = np.zeros((pairs, 128, 4), dtype=np.float32)
        rx_c, ry_c = rx[sl], ry[sl]
        wmat[:, 0:64, 0] = rx_c[0::2][:, None]
        wmat[:, 64:128, 0] = rx_c[1::2][:, None]
        wmat[:, 0:64, 1] = ry_c[0::2][:, None]
        wmat[:, 64:128, 1] = ry_c[1::2][:, None]
        in_maps.append({"x": flat, "offs": offs, "wmat": wmat,
                        "smat": smat.astype(np.float16)})
    return cfg, in_maps, last_row


N_CORES = 8
_nc_cache: dict = {}


def kernel(padded_obj: np.ndarray, positions: np.ndarray) -> np.ndarray:
    padded_obj = np.asarray(padded_obj)
    positions = np.asarray(positions)
    B, npad, _, C = padded_obj.shape
    cfg, in_maps, last_row = host_prep(
        padded_obj.reshape(B, npad, npad).astype(np.float32, copy=False),
        positions, N_CORES)

    nc = _nc_cache.get(cfg)
    if nc is None:
        nc = build_nc(cfg)
        _nc_cache[cfg] = nc

    res = run_bass_kernel_spmd(nc, in_maps, core_ids=list(range(N_CORES)))
    out = np.concatenate(
        [r["y"][:, :, :cfg.n] for r in res.results], axis=0).astype(np.float32)
    out[:, cfg.n - 1, :] = last_row
    return out.reshape(B, cfg.n, cfg.n, 1)
